# revision 21
# baseline (speedup 1.0000x reference)
"""Trainium2 Bass kernel for nn_CrossAttention (B_=64, N=512, C=128, heads=4).

Strategy: data-parallel over the B_ axis across 8 NeuronCores (8 windows per
core); parameters + relative-position-bias table replicated to every core.

Per (window, head) on device, with everything laid out transposed so that no
on-device transposes are ever needed:
    qT = (Wq*scale) @ xT          (hd=32 rows per head, 512 cols)   [PE]
    kT = Wk @ yT                                                     [PE]
    v  = yT.T @ WvT               (natural layout, k-rows x (h,hd))  [PE]
    ST = rpbT + kT.T@qT  per 128-row k-chunk (PSUM accumulate; the bias is
         injected with an identity matmul so no elementwise add is needed)
    P  = exp(ST)                  (ACT, straight out of PSUM, no max needed:
                                   |logits| < 0.5 for this problem scale)
    OT = v.T @ P, den = 1.T @ P   (column-packed matmuls, 4 heads concurrent)
    OTn = OT * bcast(1/den)       (broadcast built with a tiny K=4 matmul)
    out = OTn.T @ projwT + pb     (psum->sbuf add fuses the bias)
"""

import sys

sys.path.insert(0, "/opt/trn_rl_repo")

import numpy as np
import ml_dtypes

from contextlib import ExitStack

import concourse.bass as bass
import concourse.tile as tile
from concourse import bacc, mybir
from concourse import bass_utils

FP32 = mybir.dt.float32
BF16 = mybir.dt.bfloat16

# problem constants (hardcoded per spec: x,y are (64, 512, 128), H=W=D=8)
B_, N, C, HEADS, HD = 64, 512, 128, 4, 32
NCORES = 8
WIN = B_ // NCORES  # windows per core
POS_DIM = 8
KC = N // 128  # 4 k-chunks of 128


def _layernorm(x, g, b, eps=1e-5):
    m = x.mean(-1, keepdims=True)
    v = x.var(-1, keepdims=True)
    return (x - m) / np.sqrt(v + eps) * g + b


def _rel_pos_tables(H, W, D):
    bh = np.arange(1 - H, H)
    bw = np.arange(1 - W, W)
    bd = np.arange(1 - D, D)
    biases = np.stack(np.meshgrid(bh, bw, bd, indexing="ij")).reshape(3, -1).T
    coords = np.stack(
        np.meshgrid(np.arange(H), np.arange(W), np.arange(D), indexing="ij")
    ).reshape(3, -1)
    rel = coords[:, :, None] - coords[:, None, :]
    rel = rel.transpose(1, 2, 0).astype(np.int64)
    rel[:, :, 0] += H - 1
    rel[:, :, 1] += W - 1
    rel[:, :, 2] += D - 1
    rel[:, :, 0] *= (2 * W - 1) * (2 * D - 1)
    rel[:, :, 1] *= 2 * D - 1
    idx = rel.sum(-1)
    return biases.astype(np.float32), idx


def _build_program():
    """Build the Bass/Tile program once; returns (nc, input-name list)."""
    nc = bacc.Bacc("TRN2", target_bir_lowering=False, debug=False)

    # per-core inputs
    xT_d = nc.dram_tensor("xT", (WIN, C, N), BF16, kind="ExternalInput")
    yT_d = nc.dram_tensor("yT", (WIN, C, N), BF16, kind="ExternalInput")
    # exp(rpb), transposed per head: (h, kp, kc*512+q) bf16.  exp(S+R) is
    # computed as exp(S) * exp(R) so the bias add needs no PE/ACT work.
    rpb_d = nc.dram_tensor("expRpbT", (HEADS, 128, KC * N), BF16, kind="ExternalInput")
    wq_d = nc.dram_tensor("wqT", (C, C), BF16, kind="ExternalInput")
    wk_d = nc.dram_tensor("wkT", (C, C), BF16, kind="ExternalInput")
    wv_d = nc.dram_tensor("wvT", (C, C), BF16, kind="ExternalInput")
    pw_d = nc.dram_tensor("projwT", (C, C), BF16, kind="ExternalInput")
    pb_d = nc.dram_tensor("pb", (128, N), FP32, kind="ExternalInput")
    out_d = nc.dram_tensor("out", (WIN, N, C), FP32, kind="ExternalOutput")

    with TileCtx(nc) as tc, ExitStack() as ctx:
        const = ctx.enter_context(tc.tile_pool(name="const", bufs=1))
        xy = ctx.enter_context(tc.tile_pool(name="xy", bufs=4))
        qk_sb = ctx.enter_context(tc.tile_pool(name="qk_sb", bufs=4))
        v_pool = ctx.enter_context(tc.tile_pool(name="v_sb", bufs=2))
        p_pool = ctx.enter_context(tc.tile_pool(name="p_sb", bufs=6))
        misc = ctx.enter_context(tc.tile_pool(name="misc", bufs=2))
        outp = ctx.enter_context(tc.tile_pool(name="out_sb", bufs=2))
        mm_ps = ctx.enter_context(
            tc.tile_pool(name="mm_ps", bufs=2, space=bass.MemorySpace.PSUM)
        )
        st_ps = ctx.enter_context(
            tc.tile_pool(name="st_ps", bufs=2, space=bass.MemorySpace.PSUM)
        )

        # ---- constants, loaded once ----
        wq_sb = const.tile([C, C], BF16, tag="wq")
        wk_sb = const.tile([C, C], BF16, tag="wk")
        wv_sb = const.tile([C, C], BF16, tag="wv")
        pw_sb = const.tile([C, C], BF16, tag="pw")
        pb_sb = const.tile([128, N], FP32, tag="pb")
        rpb_sb = const.tile([128, HEADS * KC * N], BF16, tag="rpb")
        ones_sb = const.tile([128, 32], BF16, tag="ones")
        zeros_sb = const.tile([128, 128], BF16, tag="zeros")
        for dst, src in ((wq_sb, wq_d), (wk_sb, wk_d), (wv_sb, wv_d), (pw_sb, pw_d)):
            nc.sync.dma_start(dst[:], src[:])
        nc.sync.dma_start(pb_sb[:], pb_d[:])
        for h in range(HEADS):
            nc.sync.dma_start(
                rpb_sb[:, h * KC * N : (h + 1) * KC * N], rpb_d[h]
            )
        nc.vector.memset(ones_sb[:], 1.0)
        nc.vector.memset(zeros_sb[:], 0.0)

        # ---- per-window pipeline ----
        prev_exps = []
        for b in range(WIN):
            xt = xy.tile([C, N], BF16, tag="xt")
            yt = xy.tile([C, N], BF16, tag="yt")
            nc.sync.dma_start(xt[:], xT_d[b])
            nc.sync.dma_start(yt[:], yT_d[b])

            qT_ps = mm_ps.tile([128, N], FP32, tag="ps1")
            kT_ps = mm_ps.tile([128, N], FP32, tag="ps1")
            v_ps = mm_ps.tile([128, N], FP32, tag="ps1")
            nc.tensor.matmul(qT_ps[:], lhsT=wq_sb[:], rhs=xt[:], start=True, stop=True)
            nc.tensor.matmul(kT_ps[:], lhsT=wk_sb[:], rhs=yt[:], start=True, stop=True)
            for j in range(4):
                nc.tensor.matmul(
                    v_ps[:, j * 128 : (j + 1) * 128],
                    lhsT=yt[:, j * 128 : (j + 1) * 128],
                    rhs=wv_sb[:],
                    start=True,
                    stop=True,
                    skip_group_check=True,
                )
            qT_sb = qk_sb.tile([128, N], BF16, tag="qT")
            kT_sb = qk_sb.tile([128, N], BF16, tag="kT")
            v_sb = v_pool.tile([128, N], BF16, tag="v")
            # qT/kT casts on ScalarE: ACT is idle at window heads, and this
            # keeps the qkv->cast->QK chain off the busy DVE (shorter PE gap).
            nc.scalar.copy(qT_sb[:], qT_ps[:])
            nc.scalar.copy(kT_sb[:], kT_ps[:])
            nc.vector.tensor_copy(v_sb[:], v_ps[:])

            # S^T tiles + exp, per (head, half): unit is (128, 1024) = 2 k-chunks.
            # Heads processed in pairs, QK matmuls interleaved so the K=32
            # row-tiles at different tile_positions run concurrently on PE.
            p_tiles = {}
            p_muls = {}
            for hf in range(2):
                for h0 in (0, 2):
                    sts = [
                        st_ps.tile([128, 1024], FP32, tag="st", name=f"st{i}")
                        for i in range(2)
                    ]
                    for j in range(2):
                        kc = 2 * hf + j
                        sl = slice(j * 512, (j + 1) * 512)
                        for i, h in enumerate((h0, h0 + 1)):
                            # S^T = k_chunk @ qT  (K=32 row-tile at row 32h)
                            mm = nc.tensor.matmul(
                                sts[i][:, sl],
                                lhsT=kT_sb[32 * h : 32 * h + 32, kc * 128 : (kc + 1) * 128],
                                rhs=qT_sb[32 * h : 32 * h + 32, :],
                                start=True,
                                stop=True,
                                tile_position=(32 * h, 0),
                            )
                            # no-sync hints: delay each QK until the whole
                            # previous pair-unit's exps are done, so paired
                            # row-tiles land adjacent in the PE queue and
                            # run concurrently.
                            for e in prev_exps:
                                tile.add_dep_helper(
                                    mm.ins, e, False, "qk pair packing"
                                )
                    cur_exps = []
                    for i, h in enumerate((h0, h0 + 1)):
                        praw = p_pool.tile([128, 1024], BF16, tag="praw")
                        ei = nc.scalar.activation(
                            praw[:], sts[i][:], mybir.ActivationFunctionType.Exp
                        )
                        cur_exps.append(ei.ins)
                        p = p_pool.tile([128, 1024], BF16, tag="p")
                        mi = nc.vector.tensor_mul(
                            p[:],
                            praw[:],
                            rpb_sb[:, (h * KC + 2 * hf) * N : (h * KC + 2 * hf + 2) * N],
                        )
                        p_tiles[(h, hf)] = p
                        p_muls[(h, hf)] = mi.ins
                    prev_exps = cur_exps

            # O^T (col-packed, 4 heads) + denominators
            ot_ps = mm_ps.tile([128, N], FP32, tag="ps2")
            d_ps = mm_ps.tile([128, N], FP32, tag="ps2")
            # Open each accumulation bank with a zeroing matmul: clears
            # has_written for the whole bank AND writes zeros to all 128
            # partitions, so the per-head chains below can all accumulate
            # with start=False (correct under both per-element-sim and
            # whole-bank-HW has_written semantics).
            nc.tensor.matmul(
                ot_ps[:], lhsT=zeros_sb[:], rhs=rpb_sb[:, 0:N],
                start=True, stop=False, skip_group_check=True,
            )
            nc.tensor.matmul(
                d_ps[:], lhsT=zeros_sb[:], rhs=rpb_sb[:, 0:N],
                start=True, stop=False, skip_group_check=True,
            )
            # HW: start=True zeroes has_written for the WHOLE bank, so only
            # the zero-opener sets it; the per-head chains accumulate with
            # start=False (overwrite-where-clear on their first write).
            # Each 4-head group sits in one critical section so the four
            # col-strip matmuls stay adjacent on PE and run concurrently.
            for kc in range(KC):
                group_deps = [p_muls[(h, kc // 2)] for h in range(HEADS)]
                for h in range(HEADS):
                    p = p_tiles[(h, kc // 2)]
                    psl = p[:, (kc % 2) * 512 : (kc % 2 + 1) * 512]
                    mm1 = nc.tensor.matmul(
                        ot_ps[32 * h : 32 * h + 32, :],
                        lhsT=v_sb[:, kc * 128 + 32 * h : kc * 128 + 32 * h + 32],
                        rhs=psl,
                        start=False,
                        stop=(kc == KC - 1),
                        tile_position=(0, 32 * h),
                        skip_group_check=True,
                    )
                    mm2 = nc.tensor.matmul(
                        d_ps[32 * h : 32 * h + 32, :],
                        lhsT=ones_sb[:],
                        rhs=psl,
                        start=False,
                        stop=(kc == KC - 1),
                        tile_position=(0, 32 * h),
                        skip_group_check=True,
                    )
                    # no-sync hints: a 4-head group becomes schedulable only
                    # once every head's P is ready -> adjacent on PE ->
                    # concurrent col-strip execution.
                    for d in group_deps:
                        tile.add_dep_helper(mm1.ins, d, False, "pv pack")
                        tile.add_dep_helper(mm2.ins, d, False, "pv pack")

            # d_ps rows 32h..32h+31 all hold head h's denominator (the ones
            # lhsT replicates it), so 1/d_ps IS the broadcast divisor.
            # 18-bit approx is plenty: den ~ 512 +- 15%.
            invden = misc.tile([128, N], FP32, tag="invden")
            nc.vector.reciprocal_approx_fast(invden[:], d_ps[:])
            otn = misc.tile([128, N], BF16, tag="otn")
            nc.vector.tensor_mul(otn[:], ot_ps[:], invden[:])

            # proj + bias
            pr_ps = mm_ps.tile([128, N], FP32, tag="ps2")
            for s in range(4):
                nc.tensor.matmul(
                    pr_ps[:, s * 128 : (s + 1) * 128],
                    lhsT=otn[:, s * 128 : (s + 1) * 128],
                    rhs=pw_sb[:],
                    start=True,
                    stop=True,
                )
            ot = outp.tile([128, N], FP32, tag="out")
            nc.vector.tensor_add(ot[:], pr_ps[:], pb_sb[:])
            nc.sync.dma_start(
                out_d[b].rearrange("(s p) c -> p s c", p=128),
                ot.rearrange("p (s c) -> p s c", s=4),
            )
    nc.compile()
    return nc


def TileCtx(nc):
    return tile.TileContext(nc)


_CACHE = {}


def _get_program():
    if "nc" not in _CACHE:
        _CACHE["nc"] = _build_program()
    return _CACHE["nc"]


def _host_prep(x, y, H, W, D, qkv_w, qkv_b, proj_w, proj_b,
               pos_proj_w, pos_proj_b, ln1_g, ln1_b, p1_w, p1_b,
               ln2_g, ln2_b, p2_w, p2_b, ln3_g, ln3_b, p3_w, p3_b):
    """Numpy-only prep: layout transforms, weight folding, pos-bias table."""
    scale = HD ** -0.5
    bf = ml_dtypes.bfloat16

    xT = np.ascontiguousarray(x.transpose(0, 2, 1)).astype(bf)  # (B_, C, N)
    yT = np.ascontiguousarray(y.transpose(0, 2, 1)).astype(bf)

    wqT = np.ascontiguousarray((qkv_w[0:C] * scale).T).astype(bf)
    wkT = np.ascontiguousarray(qkv_w[C : 2 * C].T).astype(bf)
    wvT = np.ascontiguousarray(qkv_w[2 * C : 3 * C].T).astype(bf)
    projwT = np.ascontiguousarray(proj_w.T).astype(bf)

    # pos-bias MLP (tiny: 3375x8), exact fp32 replica of the reference math
    biases, idx = _rel_pos_tables(int(H), int(W), int(D))
    pos = biases @ pos_proj_w.T + pos_proj_b
    pos = np.maximum(_layernorm(pos, ln1_g, ln1_b), 0) @ p1_w.T + p1_b
    pos = np.maximum(_layernorm(pos, ln2_g, ln2_b), 0) @ p2_w.T + p2_b
    pos = np.maximum(_layernorm(pos, ln3_g, ln3_b), 0) @ p3_w.T + p3_b  # (T, h)
    rpb = pos[idx.reshape(-1)].reshape(N, N, HEADS)  # [q, k, h]
    # q/k bias terms of (q+bq)(k+bk): folded into the additive bias table.
    # rpb'[q,k,h] = rpb + (bq_h . k_h[k]) + (bk_h . q_h[q]) + bq_h.bk_h is only
    # exact per-window; the reference uses qkv_b = 0 so skip unless nonzero.
    bq = qkv_b[0:C]
    bk = qkv_b[C : 2 * C]
    if np.any(bq) or np.any(bk):
        raise NotImplementedError("nonzero qkv bias not supported")
    rpbT = np.exp(rpb.transpose(2, 1, 0))  # [h, k, q] -> exp for mult-bias
    rpbT = np.ascontiguousarray(
        rpbT.reshape(HEADS, KC, 128, N).transpose(0, 2, 1, 3).reshape(HEADS, 128, KC * N)
    ).astype(bf)

    pb_full = proj_b + qkv_b[2 * C : 3 * C] @ proj_w.T  # fold v bias thru proj
    pb = np.tile(pb_full[None, :], (128, 4)).astype(np.float32)  # (128, 512)

    return xT, yT, rpbT, wqT, wkT, wvT, projwT, pb


def kernel(**inputs):
    inputs = {k: np.asarray(v) if not np.isscalar(v) else v for k, v in inputs.items()}
    x = np.asarray(inputs["x"], np.float32)
    assert x.shape == (B_, N, C)
    xT, yT, rpbT, wqT, wkT, wvT, projwT, pb = _host_prep(
        np.asarray(inputs["x"], np.float32),
        np.asarray(inputs["y"], np.float32),
        inputs["H"], inputs["W"], inputs["D"],
        np.asarray(inputs["qkv_w"], np.float32),
        np.asarray(inputs["qkv_b"], np.float32),
        np.asarray(inputs["proj_w"], np.float32),
        np.asarray(inputs["proj_b"], np.float32),
        np.asarray(inputs["pos_proj_w"], np.float32),
        np.asarray(inputs["pos_proj_b"], np.float32),
        np.asarray(inputs["ln1_g"], np.float32), np.asarray(inputs["ln1_b"], np.float32),
        np.asarray(inputs["p1_w"], np.float32), np.asarray(inputs["p1_b"], np.float32),
        np.asarray(inputs["ln2_g"], np.float32), np.asarray(inputs["ln2_b"], np.float32),
        np.asarray(inputs["p2_w"], np.float32), np.asarray(inputs["p2_b"], np.float32),
        np.asarray(inputs["ln3_g"], np.float32), np.asarray(inputs["ln3_b"], np.float32),
        np.asarray(inputs["p3_w"], np.float32), np.asarray(inputs["p3_b"], np.float32),
    )

    nc = _get_program()
    in_maps = []
    for c in range(NCORES):
        sl = slice(c * WIN, (c + 1) * WIN)
        in_maps.append(
            {
                "xT": xT[sl],
                "yT": yT[sl],
                "expRpbT": rpbT,
                "wqT": wqT,
                "wkT": wkT,
                "wvT": wvT,
                "projwT": projwT,
                "pb": pb,
            }
        )
    kwargs = {}
    if PROFILE:
        kwargs = dict(trace=True, **PROFILE_KWARGS)
    res = bass_utils.run_bass_kernel_spmd(
        nc, in_maps, core_ids=list(range(NCORES)), **kwargs
    )
    global LAST_EXEC_NS, LAST_RESULTS
    LAST_EXEC_NS = res.exec_time_ns
    LAST_RESULTS = res
    out = np.concatenate([np.asarray(r["out"]) for r in res.results], axis=0)
    return out.astype(np.float32)


PROFILE = False
PROFILE_KWARGS = {}
LAST_EXEC_NS = None
LAST_RESULTS = None


if __name__ == "__main__":
    # smoke test with random data
    rng = np.random.default_rng(0)
    demo = {
        "x": rng.standard_normal((B_, N, C), np.float32),
        "y": rng.standard_normal((B_, N, C), np.float32),
        "H": 8, "W": 8, "D": 8,
        "qkv_w": rng.standard_normal((3 * C, C), np.float32) * 0.02,
        "qkv_b": np.zeros(3 * C, np.float32),
        "proj_w": rng.standard_normal((C, C), np.float32) * 0.02,
        "proj_b": np.zeros(C, np.float32),
        "pos_proj_w": rng.standard_normal((POS_DIM, 3), np.float32) * 0.02,
        "pos_proj_b": np.zeros(POS_DIM, np.float32),
        "ln1_g": np.ones(POS_DIM, np.float32), "ln1_b": np.zeros(POS_DIM, np.float32),
        "p1_w": rng.standard_normal((POS_DIM, POS_DIM), np.float32) * 0.02,
        "p1_b": np.zeros(POS_DIM, np.float32),
        "ln2_g": np.ones(POS_DIM, np.float32), "ln2_b": np.zeros(POS_DIM, np.float32),
        "p2_w": rng.standard_normal((POS_DIM, POS_DIM), np.float32) * 0.02,
        "p2_b": np.zeros(POS_DIM, np.float32),
        "ln3_g": np.ones(POS_DIM, np.float32), "ln3_b": np.zeros(POS_DIM, np.float32),
        "p3_w": rng.standard_normal((HEADS, POS_DIM), np.float32) * 0.02,
        "p3_b": np.zeros(HEADS, np.float32),
    }
    out = kernel(**demo)
    print("kernel out:", out.shape, out.dtype, np.abs(out).max())
